# revision 27
# baseline (speedup 1.0000x reference)
"""MoE grouped linear (DMoELinear) on 8 Trainium2 NeuronCores.

Expert-parallel sharding: tokens are sorted by expert id, so expert e's
tokens form one contiguous slice. Core e receives expert e's tokens
(padded to a uniform capacity C = max group size, so all cores run one
SPMD NEFF), expert e's weight and bias, and computes
    yT_e = (x_e @ W_e.T).T.bf16 + b_e.bf16
with the weight block as the stationary matmul operand and tokens as
the moving free dim (so C needs no 128-alignment). The weight is laid
out db-major ([16 output blocks] x [8 k-tiles] x [128k x 128dout]) so
DMA arrival order matches PE consumption order. The bias add fuses into
the PSUM-evacuation op as a per-partition scalar. The host does all
routing/gather in numpy.
"""

import numpy as np
import ml_dtypes

N_TOK, D_IN, D_OUT, N_EXP = 8192, 1024, 2048, 8
N_CORES = 8
P = 128
NFREE = 512  # max matmul moving free dim (one PSUM bank of f32)

BF16 = ml_dtypes.bfloat16

_nc_cache: dict[int, object] = {}


def _chunks(C):
    out = []
    off = 0
    while off < C:
        cw = min(NFREE, C - off)
        out.append((off, cw))
        off += cw
    return out


def _build_bass(C: int):
    """Emit the per-core Bass/Tile kernel for token capacity C."""
    import concourse.bass as bass  # noqa: F401  (registers engines)
    import concourse.mybir as mybir
    import concourse.tile as tile
    from concourse import bacc

    dt = mybir.dt
    KT = D_IN // P      # 8 contraction tiles
    DB = D_OUT // P     # 16 output-row blocks
    chunks = _chunks(C)

    nc = bacc.Bacc("TRN2", target_bir_lowering=False)

    # x: [ki][128k x C] halves split across both HWDGE rings.
    xT_d = nc.dram_tensor("xT", [D_IN, C], dt.bfloat16, kind="ExternalInput")
    # w: db-major, partition-contiguous: row (db*128+p) holds all KT
    # 128-wide k-slices for output block db, SBUF partition p.
    w_d = nc.dram_tensor("wdb", [DB * P, KT * P], dt.bfloat16, kind="ExternalInput")
    bias_d = nc.dram_tensor("biasp", [P, DB], dt.float32, kind="ExternalInput")
    y_d = nc.dram_tensor("yT", [D_OUT, C], dt.bfloat16, kind="ExternalOutput")

    half = C // 2

    with tile.TileContext(nc) as tc:
        with (
            tc.tile_pool(name="persist", bufs=1) as ppool,
            tc.tile_pool(name="yout", bufs=3) as ypool,
            tc.tile_pool(name="psum", bufs=8, space="PSUM") as pspool,
        ):
            # DMA emission order = PE consumption order: w_db0/w_db1 lead
            # the two HWDGE rings, then x k-tiles (halves split across
            # rings), then the remaining w blocks alternating.
            w_tiles = [
                ppool.tile([P, KT * P], dt.bfloat16, name=f"w{db}", tag=f"w{db}")
                for db in range(DB)
            ]

            def w_dma(db):
                eng = nc.sync if db % 2 == 0 else nc.scalar
                eng.dma_start(w_tiles[db][:], w_d[db * P:(db + 1) * P, :])

            # Critical path to the first matmul (db0, k0, chunk0): its two
            # operand slices go first as small DMAs (earlier last-byte =>
            # earlier completion receipt), then the rest in consumption
            # order interleaved across the two rings.
            x_tiles = [
                ppool.tile([P, C], dt.bfloat16, name=f"x{ki}", tag=f"x{ki}")
                for ki in range(KT)
            ]
            nc.sync.dma_start(w_tiles[0][:, :P], w_d[:P, :P])
            nc.sync.dma_start(x_tiles[0][:, :half], xT_d[:P, :half])
            nc.sync.dma_start(w_tiles[0][:, P:], w_d[:P, P:])
            w_dma(1)  # scalar ring
            nc.scalar.dma_start(x_tiles[0][:, half:], xT_d[:P, half:])
            for ki in range(1, KT):
                nc.sync.dma_start(
                    x_tiles[ki][:, :half], xT_d[ki * P:(ki + 1) * P, :half]
                )
                nc.scalar.dma_start(
                    x_tiles[ki][:, half:], xT_d[ki * P:(ki + 1) * P, half:]
                )
            w_dma(2)
            w_dma(3)
            bt = ppool.tile([P, DB], dt.float32, name="bias", tag="bias")
            nc.sync.dma_start(bt[:], bias_d[:])
            for db in range(4, DB):
                w_dma(db)

            # Warm the PE's HAM clock gate with dummy matmuls on a zeroed
            # scratch tile while the input DMAs stream in (~3.4us of PE
            # activity flips the clock gate from 1.2GHz to 2.4GHz). Small
            # N=128 matmuls so real work waits at most ~110ns once its
            # inputs land.
            warm = ppool.tile([P, P], dt.bfloat16, name="warm", tag="warm")
            nc.vector.memset(warm[:], 0.0)
            wps = pspool.tile([P, P], dt.float32, name="wps", tag="ps")
            for _ in range(30):
                nc.tensor.matmul(wps[:], warm[:], warm[:], start=True, stop=True)

            all_psums = {}

            def alloc_psums(db):
                all_psums[db] = [
                    pspool.tile([P, NFREE], dt.float32, name=f"ps{db}_{j}", tag="ps")
                    for j in range(len(chunks))
                ]

            def emit_mms(db, ki):
                lhsT = w_tiles[db][:, ki * P:(ki + 1) * P]
                for j, (off, cw) in enumerate(chunks):
                    nc.tensor.matmul(
                        all_psums[db][j][:, :cw],
                        lhsT,
                        x_tiles[ki][:, off:off + cw],
                        start=(ki == 0),
                        stop=(ki == KT - 1),
                    )

            # First two db blocks interleaved by k-step: during the x DMA
            # trickle the PE consumes each freshly-landed x tile twice,
            # staying busier than a single db's worth of work would.
            alloc_psums(0)
            alloc_psums(1)
            for ki in range(KT):
                emit_mms(0, ki)
                emit_mms(1, ki)

            ep = 0
            for db in range(DB):
                if db >= 2:
                    alloc_psums(db)
                    for ki in range(KT):
                        emit_mms(db, ki)
                psums = all_psums[db]
                ysb = ypool.tile([P, C], dt.bfloat16, name="ysb", tag="ysb")
                bias_col = bt[:, db:db + 1]
                for j, (off, cw) in enumerate(chunks):
                    # fused PSUM->bf16 cast + per-partition bias add,
                    # alternating ACT / DVE so neither engine bottlenecks
                    if ep % 2 == 0:
                        nc.scalar.add(ysb[:, off:off + cw], psums[j][:, :cw], bias_col)
                    else:
                        nc.vector.tensor_scalar_add(
                            ysb[:, off:off + cw], psums[j][:, :cw], bias_col
                        )
                    ep += 1
                if db >= DB - 2:
                    # tail blocks: per-chunk DMAs on alternating rings so the
                    # final transfer (smallest chunk) lands ASAP
                    for j, (off, cw) in enumerate(chunks):
                        eng = nc.sync if (db + j) % 2 == 0 else nc.scalar
                        eng.dma_start(
                            y_d[db * P:(db + 1) * P, off:off + cw],
                            ysb[:, off:off + cw],
                        )
                else:
                    eng = nc.sync if db % 2 == 0 else nc.scalar
                    eng.dma_start(y_d[db * P:(db + 1) * P, :], ysb[:])

    nc.compile()
    return nc


def _run_spmd(in_maps, C, trace=False, trace_cores=None):
    from concourse.bass_utils import run_bass_kernel_spmd

    nc = _nc_cache.get(C)
    if nc is None:
        nc = _build_bass(C)
        _nc_cache[C] = nc
    return run_bass_kernel_spmd(
        nc,
        in_maps,
        core_ids=list(range(N_CORES)),
        trace=trace,
        trace_cores=trace_cores,
    )


def _prepare(x, weight, bias, ids_sorted):
    """Host-side routing: returns (in_maps, C, counts, starts)."""
    x = np.asarray(x)
    weight = np.asarray(weight)
    bias = np.asarray(bias)
    ids = np.asarray(ids_sorted)

    counts = np.bincount(ids, minlength=N_EXP).astype(np.int64)
    starts = np.zeros(N_EXP, dtype=np.int64)
    starts[1:] = np.cumsum(counts)[:-1]
    C = max(int(counts.max()), 2)
    C += C % 2  # keep the half-split even

    xb = x.astype(BF16)
    in_maps = []
    for e in range(N_EXP):
        n_e = int(counts[e])
        xeT = np.zeros((D_IN, C), dtype=BF16)
        if n_e:
            xeT[:, :n_e] = xb[starts[e]:starts[e] + n_e].T
        # db-major weight: row (db*128+p) = [w.T[kt*128+p, db*128+d] for kt, d]
        weT = weight[e].T.astype(BF16)  # [d_in, d_out]
        wdb = np.ascontiguousarray(
            weT.reshape(D_IN // P, P, D_OUT // P, P).transpose(2, 1, 0, 3)
        ).reshape(D_OUT, D_IN)
        bp = np.ascontiguousarray(
            bias[e].astype(BF16).astype(np.float32).reshape(D_OUT // P, P).T
        )
        in_maps.append({"xT": xeT, "wdb": wdb, "biasp": bp})
    return in_maps, C, counts, starts


def _assemble(results, counts, starts):
    out = np.empty((N_TOK, D_OUT), dtype=BF16)
    for e in range(N_EXP):
        n_e = int(counts[e])
        if n_e:
            out[starts[e]:starts[e] + n_e] = results[e]["yT"][:, :n_e].T
    return out


def kernel(x, weight, bias, ids_sorted):
    in_maps, C, counts, starts = _prepare(x, weight, bias, ids_sorted)
    res = _run_spmd(in_maps, C)
    return _assemble(res.results, counts, starts)


# revision 29
# speedup vs baseline: 1.0006x; 1.0006x over previous
"""MoE grouped linear (DMoELinear) on 8 Trainium2 NeuronCores.

Expert-parallel sharding: tokens are sorted by expert id, so expert e's
tokens form one contiguous slice. Core e receives expert e's tokens
(padded to a uniform capacity C = max group size, so all cores run one
SPMD NEFF), expert e's weight and bias, and computes
    yT_e = (x_e @ W_e.T).T.bf16 + b_e.bf16
with the weight block as the stationary matmul operand and tokens as
the moving free dim (so C needs no 128-alignment). The weight is laid
out db-major ([16 output blocks] x [8 k-tiles] x [128k x 128dout]) so
DMA arrival order matches PE consumption order. The bias add fuses into
the PSUM-evacuation op as a per-partition scalar. The host does all
routing/gather in numpy.
"""

import numpy as np
import ml_dtypes

N_TOK, D_IN, D_OUT, N_EXP = 8192, 1024, 2048, 8
N_CORES = 8
P = 128
NFREE = 512  # max matmul moving free dim (one PSUM bank of f32)

BF16 = ml_dtypes.bfloat16

_nc_cache: dict[int, object] = {}


def _chunks(C):
    out = []
    off = 0
    while off < C:
        cw = min(NFREE, C - off)
        out.append((off, cw))
        off += cw
    return out


def _build_bass(C: int):
    """Emit the per-core Bass/Tile kernel for token capacity C."""
    import concourse.bass as bass  # noqa: F401  (registers engines)
    import concourse.mybir as mybir
    import concourse.tile as tile
    from concourse import bacc

    dt = mybir.dt
    KT = D_IN // P      # 8 contraction tiles
    DB = D_OUT // P     # 16 output-row blocks
    chunks = _chunks(C)

    nc = bacc.Bacc("TRN2", target_bir_lowering=False)

    # x: [ki][128k x C] halves split across both HWDGE rings.
    xT_d = nc.dram_tensor("xT", [D_IN, C], dt.bfloat16, kind="ExternalInput")
    # w: db-major, partition-contiguous: row (db*128+p) holds all KT
    # 128-wide k-slices for output block db, SBUF partition p.
    w_d = nc.dram_tensor("wdb", [DB * P, KT * P], dt.bfloat16, kind="ExternalInput")
    bias_d = nc.dram_tensor("biasp", [P, DB], dt.float32, kind="ExternalInput")
    y_d = nc.dram_tensor("yT", [D_OUT, C], dt.bfloat16, kind="ExternalOutput")

    half = C // 2

    with tile.TileContext(nc) as tc:
        with (
            tc.tile_pool(name="persist", bufs=1) as ppool,
            tc.tile_pool(name="yout", bufs=3) as ypool,
            tc.tile_pool(name="psum", bufs=8, space="PSUM") as pspool,
        ):
            # DMA emission order = PE consumption order: w_db0/w_db1 lead
            # the two HWDGE rings, then x k-tiles (halves split across
            # rings), then the remaining w blocks alternating.
            w_tiles = [
                ppool.tile([P, KT * P], dt.bfloat16, name=f"w{db}", tag=f"w{db}")
                for db in range(DB)
            ]

            def w_dma(db):
                eng = nc.sync if db % 2 == 0 else nc.scalar
                eng.dma_start(w_tiles[db][:], w_d[db * P:(db + 1) * P, :])

            # Critical path to the first matmul (db0, k0, chunk0): its two
            # operand slices go first as small DMAs (earlier last-byte =>
            # earlier completion receipt), then the rest in consumption
            # order interleaved across the two rings.
            x_tiles = [
                ppool.tile([P, C], dt.bfloat16, name=f"x{ki}", tag=f"x{ki}")
                for ki in range(KT)
            ]
            nc.sync.dma_start(w_tiles[0][:, :P], w_d[:P, :P])
            nc.sync.dma_start(x_tiles[0][:, :half], xT_d[:P, :half])
            nc.sync.dma_start(w_tiles[0][:, P:], w_d[:P, P:])
            w_dma(1)  # scalar ring
            nc.scalar.dma_start(x_tiles[0][:, half:], xT_d[:P, half:])
            # w2-w5 interleave INTO the x stream (not after it): on the
            # in-order rings a w tile issued behind all eight x halves
            # lands ~1us after its first consumer needs it (trace: the
            # db2/db4 k0 matmuls stall on w arrival).
            for ki in range(1, KT):
                nc.sync.dma_start(
                    x_tiles[ki][:, :half], xT_d[ki * P:(ki + 1) * P, :half]
                )
                nc.scalar.dma_start(
                    x_tiles[ki][:, half:], xT_d[ki * P:(ki + 1) * P, half:]
                )
                if ki == 2:
                    w_dma(2)
                    w_dma(3)
                elif ki == 4:
                    w_dma(4)
                    w_dma(5)
            bt = ppool.tile([P, DB], dt.float32, name="bias", tag="bias")
            nc.sync.dma_start(bt[:], bias_d[:])
            for db in range(6, DB):
                w_dma(db)

            # Warm the PE's HAM clock gate with dummy matmuls on a zeroed
            # scratch tile while the input DMAs stream in (~3.4us of PE
            # activity flips the clock gate from 1.2GHz to 2.4GHz). Small
            # N=128 matmuls so real work waits at most ~110ns once its
            # inputs land.
            warm = ppool.tile([P, P], dt.bfloat16, name="warm", tag="warm")
            nc.vector.memset(warm[:], 0.0)
            wps = pspool.tile([P, P], dt.float32, name="wps", tag="ps")
            for _ in range(44):
                nc.tensor.matmul(wps[:], warm[:], warm[:], start=True, stop=True)

            all_psums = {}

            def alloc_psums(db):
                all_psums[db] = [
                    pspool.tile([P, NFREE], dt.float32, name=f"ps{db}_{j}", tag="ps")
                    for j in range(len(chunks))
                ]

            def emit_mms(db, ki):
                lhsT = w_tiles[db][:, ki * P:(ki + 1) * P]
                for j, (off, cw) in enumerate(chunks):
                    nc.tensor.matmul(
                        all_psums[db][j][:, :cw],
                        lhsT,
                        x_tiles[ki][:, off:off + cw],
                        start=(ki == 0),
                        stop=(ki == KT - 1),
                    )

            # First two db blocks interleaved by k-step: during the x DMA
            # trickle the PE consumes each freshly-landed x tile twice,
            # staying busier than a single db's worth of work would.
            alloc_psums(0)
            alloc_psums(1)
            for ki in range(KT):
                emit_mms(0, ki)
                emit_mms(1, ki)

            ep = 0
            for db in range(DB):
                if db >= 2:
                    alloc_psums(db)
                    for ki in range(KT):
                        emit_mms(db, ki)
                psums = all_psums[db]
                ysb = ypool.tile([P, C], dt.bfloat16, name="ysb", tag="ysb")
                bias_col = bt[:, db:db + 1]
                for j, (off, cw) in enumerate(chunks):
                    # fused PSUM->bf16 cast + per-partition bias add,
                    # alternating ACT / DVE so neither engine bottlenecks
                    if ep % 2 == 0:
                        nc.scalar.add(ysb[:, off:off + cw], psums[j][:, :cw], bias_col)
                    else:
                        nc.vector.tensor_scalar_add(
                            ysb[:, off:off + cw], psums[j][:, :cw], bias_col
                        )
                    ep += 1
                if db >= DB - 2:
                    # tail blocks: per-chunk DMAs on alternating rings so the
                    # final transfer (smallest chunk) lands ASAP
                    for j, (off, cw) in enumerate(chunks):
                        eng = nc.sync if (db + j) % 2 == 0 else nc.scalar
                        eng.dma_start(
                            y_d[db * P:(db + 1) * P, off:off + cw],
                            ysb[:, off:off + cw],
                        )
                else:
                    eng = nc.sync if db % 2 == 0 else nc.scalar
                    eng.dma_start(y_d[db * P:(db + 1) * P, :], ysb[:])

    nc.compile()
    return nc


def _run_spmd(in_maps, C, trace=False, trace_cores=None):
    from concourse.bass_utils import run_bass_kernel_spmd

    nc = _nc_cache.get(C)
    if nc is None:
        nc = _build_bass(C)
        _nc_cache[C] = nc
    return run_bass_kernel_spmd(
        nc,
        in_maps,
        core_ids=list(range(N_CORES)),
        trace=trace,
        trace_cores=trace_cores,
    )


def _prepare(x, weight, bias, ids_sorted):
    """Host-side routing: returns (in_maps, C, counts, starts)."""
    x = np.asarray(x)
    weight = np.asarray(weight)
    bias = np.asarray(bias)
    ids = np.asarray(ids_sorted)

    counts = np.bincount(ids, minlength=N_EXP).astype(np.int64)
    starts = np.zeros(N_EXP, dtype=np.int64)
    starts[1:] = np.cumsum(counts)[:-1]
    C = max(int(counts.max()), 2)
    C += C % 2  # keep the half-split even

    xb = x.astype(BF16)
    in_maps = []
    for e in range(N_EXP):
        n_e = int(counts[e])
        xeT = np.zeros((D_IN, C), dtype=BF16)
        if n_e:
            xeT[:, :n_e] = xb[starts[e]:starts[e] + n_e].T
        # db-major weight: row (db*128+p) = [w.T[kt*128+p, db*128+d] for kt, d]
        weT = weight[e].T.astype(BF16)  # [d_in, d_out]
        wdb = np.ascontiguousarray(
            weT.reshape(D_IN // P, P, D_OUT // P, P).transpose(2, 1, 0, 3)
        ).reshape(D_OUT, D_IN)
        bp = np.ascontiguousarray(
            bias[e].astype(BF16).astype(np.float32).reshape(D_OUT // P, P).T
        )
        in_maps.append({"xT": xeT, "wdb": wdb, "biasp": bp})
    return in_maps, C, counts, starts


def _assemble(results, counts, starts):
    out = np.empty((N_TOK, D_OUT), dtype=BF16)
    for e in range(N_EXP):
        n_e = int(counts[e])
        if n_e:
            out[starts[e]:starts[e] + n_e] = results[e]["yT"][:, :n_e].T
    return out


def kernel(x, weight, bias, ids_sorted):
    in_maps, C, counts, starts = _prepare(x, weight, bias, ids_sorted)
    res = _run_spmd(in_maps, C)
    return _assemble(res.results, counts, starts)


# revision 30
# speedup vs baseline: 1.0239x; 1.0234x over previous
"""MoE grouped linear (DMoELinear) on 8 Trainium2 NeuronCores.

Expert-parallel sharding: tokens are sorted by expert id, so expert e's
tokens form one contiguous slice. Core e receives expert e's tokens
(padded to a uniform capacity C = max group size, so all cores run one
SPMD NEFF), expert e's weight and bias, and computes
    yT_e = (x_e @ W_e.T).T.bf16 + b_e.bf16
with the weight block as the stationary matmul operand and tokens as
the moving free dim (so C needs no 128-alignment). The weight is laid
out db-major ([16 output blocks] x [8 k-tiles] x [128k x 128dout]) so
DMA arrival order matches PE consumption order. The bias add fuses into
the PSUM-evacuation op as a per-partition scalar. The host does all
routing/gather in numpy.
"""

import numpy as np
import ml_dtypes

N_TOK, D_IN, D_OUT, N_EXP = 8192, 1024, 2048, 8
N_CORES = 8
P = 128
NFREE = 512  # max matmul moving free dim (one PSUM bank of f32)

BF16 = ml_dtypes.bfloat16

_nc_cache: dict[int, object] = {}


def _chunks(C):
    out = []
    off = 0
    while off < C:
        cw = min(NFREE, C - off)
        out.append((off, cw))
        off += cw
    return out


def _build_bass(C: int):
    """Emit the per-core Bass/Tile kernel for token capacity C."""
    import concourse.bass as bass  # noqa: F401  (registers engines)
    import concourse.mybir as mybir
    import concourse.tile as tile
    from concourse import bacc

    dt = mybir.dt
    KT = D_IN // P      # 8 contraction tiles
    DB = D_OUT // P     # 16 output-row blocks
    chunks = _chunks(C)

    nc = bacc.Bacc("TRN2", target_bir_lowering=False)

    # x: [ki][128k x C] halves split across both HWDGE rings.
    xT_d = nc.dram_tensor("xT", [D_IN, C], dt.bfloat16, kind="ExternalInput")
    # w: db-major, partition-contiguous: row (db*128+p) holds all KT
    # 128-wide k-slices for output block db, SBUF partition p.
    w_d = nc.dram_tensor("wdb", [DB * P, KT * P], dt.bfloat16, kind="ExternalInput")
    bias_d = nc.dram_tensor("biasp", [P, DB], dt.float32, kind="ExternalInput")
    y_d = nc.dram_tensor("yT", [D_OUT, C], dt.bfloat16, kind="ExternalOutput")

    half = C // 2

    with tile.TileContext(nc) as tc:
        with (
            tc.tile_pool(name="persist", bufs=1) as ppool,
            tc.tile_pool(name="yout", bufs=3) as ypool,
            tc.tile_pool(name="psum", bufs=8, space="PSUM") as pspool,
        ):
            # DMA emission order = PE consumption order: w_db0/w_db1 lead
            # the two HWDGE rings, then x k-tiles (halves split across
            # rings), then the remaining w blocks alternating.
            w_tiles = [
                ppool.tile([P, KT * P], dt.bfloat16, name=f"w{db}", tag=f"w{db}")
                for db in range(DB)
            ]

            def w_dma(db):
                eng = nc.sync if db % 2 == 0 else nc.scalar
                eng.dma_start(w_tiles[db][:], w_d[db * P:(db + 1) * P, :])

            # Critical path to the first matmul (db0, k0, chunk0): its two
            # operand slices go first as small DMAs (earlier last-byte =>
            # earlier completion receipt), then the rest in consumption
            # order interleaved across the two rings.
            x_tiles = [
                ppool.tile([P, C], dt.bfloat16, name=f"x{ki}", tag=f"x{ki}")
                for ki in range(KT)
            ]
            nc.sync.dma_start(w_tiles[0][:, :P], w_d[:P, :P])
            nc.sync.dma_start(x_tiles[0][:, :half], xT_d[:P, :half])
            nc.sync.dma_start(w_tiles[0][:, P:], w_d[:P, P:])
            w_dma(1)  # scalar ring
            nc.scalar.dma_start(x_tiles[0][:, half:], xT_d[:P, half:])
            for ki in range(1, KT):
                nc.sync.dma_start(
                    x_tiles[ki][:, :half], xT_d[ki * P:(ki + 1) * P, :half]
                )
                nc.scalar.dma_start(
                    x_tiles[ki][:, half:], xT_d[ki * P:(ki + 1) * P, half:]
                )
            w_dma(2)
            w_dma(3)
            bt = ppool.tile([P, DB], dt.float32, name="bias", tag="bias")
            nc.sync.dma_start(bt[:], bias_d[:])
            for db in range(4, DB):
                w_dma(db)

            # Warm the PE's HAM clock gate with dummy matmuls on a zeroed
            # scratch tile while the input DMAs stream in (~3.4us of PE
            # activity flips the clock gate from 1.2GHz to 2.4GHz). Small
            # N=128 matmuls so real work waits at most ~110ns once its
            # inputs land.
            warm = ppool.tile([P, P], dt.bfloat16, name="warm", tag="warm")
            nc.vector.memset(warm[:], 0.0)
            wps = pspool.tile([P, P], dt.float32, name="wps", tag="ps")
            for _ in range(44):
                nc.tensor.matmul(wps[:], warm[:], warm[:], start=True, stop=True)

            all_psums = {}

            def alloc_psums(db):
                all_psums[db] = [
                    pspool.tile([P, NFREE], dt.float32, name=f"ps{db}_{j}", tag="ps")
                    for j in range(len(chunks))
                ]

            def emit_mms(db, ki):
                lhsT = w_tiles[db][:, ki * P:(ki + 1) * P]
                for j, (off, cw) in enumerate(chunks):
                    nc.tensor.matmul(
                        all_psums[db][j][:, :cw],
                        lhsT,
                        x_tiles[ki][:, off:off + cw],
                        start=(ki == 0),
                        stop=(ki == KT - 1),
                    )

            # First two db blocks interleaved by k-step: during the x DMA
            # trickle the PE consumes each freshly-landed x tile twice,
            # staying busier than a single db's worth of work would.
            alloc_psums(0)
            alloc_psums(1)
            for ki in range(KT):
                emit_mms(0, ki)
                emit_mms(1, ki)

            ep = 0
            for db in range(DB):
                if db >= 2:
                    alloc_psums(db)
                    for ki in range(KT):
                        emit_mms(db, ki)
                psums = all_psums[db]
                ysb = ypool.tile([P, C], dt.bfloat16, name="ysb", tag="ysb")
                bias_col = bt[:, db:db + 1]
                for j, (off, cw) in enumerate(chunks):
                    # fused PSUM->bf16 cast + per-partition bias add,
                    # alternating ACT / DVE so neither engine bottlenecks
                    if ep % 2 == 0:
                        nc.scalar.add(ysb[:, off:off + cw], psums[j][:, :cw], bias_col)
                    else:
                        nc.vector.tensor_scalar_add(
                            ysb[:, off:off + cw], psums[j][:, :cw], bias_col
                        )
                    ep += 1
                if db >= DB - 2:
                    # tail blocks: per-chunk DMAs on alternating rings so the
                    # final transfer (smallest chunk) lands ASAP
                    for j, (off, cw) in enumerate(chunks):
                        eng = nc.sync if (db + j) % 2 == 0 else nc.scalar
                        eng.dma_start(
                            y_d[db * P:(db + 1) * P, off:off + cw],
                            ysb[:, off:off + cw],
                        )
                else:
                    eng = nc.sync if db % 2 == 0 else nc.scalar
                    eng.dma_start(y_d[db * P:(db + 1) * P, :], ysb[:])

    nc.compile()
    return nc


def _run_spmd(in_maps, C, trace=False, trace_cores=None):
    from concourse.bass_utils import run_bass_kernel_spmd

    nc = _nc_cache.get(C)
    if nc is None:
        nc = _build_bass(C)
        _nc_cache[C] = nc
    return run_bass_kernel_spmd(
        nc,
        in_maps,
        core_ids=list(range(N_CORES)),
        trace=trace,
        trace_cores=trace_cores,
    )


def _prepare(x, weight, bias, ids_sorted):
    """Host-side routing: returns (in_maps, C, counts, starts)."""
    x = np.asarray(x)
    weight = np.asarray(weight)
    bias = np.asarray(bias)
    ids = np.asarray(ids_sorted)

    counts = np.bincount(ids, minlength=N_EXP).astype(np.int64)
    starts = np.zeros(N_EXP, dtype=np.int64)
    starts[1:] = np.cumsum(counts)[:-1]
    C = max(int(counts.max()), 2)
    C += C % 2  # keep the half-split even

    xb = x.astype(BF16)
    in_maps = []
    for e in range(N_EXP):
        n_e = int(counts[e])
        xeT = np.zeros((D_IN, C), dtype=BF16)
        if n_e:
            xeT[:, :n_e] = xb[starts[e]:starts[e] + n_e].T
        # db-major weight: row (db*128+p) = [w.T[kt*128+p, db*128+d] for kt, d]
        weT = weight[e].T.astype(BF16)  # [d_in, d_out]
        wdb = np.ascontiguousarray(
            weT.reshape(D_IN // P, P, D_OUT // P, P).transpose(2, 1, 0, 3)
        ).reshape(D_OUT, D_IN)
        bp = np.ascontiguousarray(
            bias[e].astype(BF16).astype(np.float32).reshape(D_OUT // P, P).T
        )
        in_maps.append({"xT": xeT, "wdb": wdb, "biasp": bp})
    return in_maps, C, counts, starts


def _assemble(results, counts, starts):
    out = np.empty((N_TOK, D_OUT), dtype=BF16)
    for e in range(N_EXP):
        n_e = int(counts[e])
        if n_e:
            out[starts[e]:starts[e] + n_e] = results[e]["yT"][:, :n_e].T
    return out


def kernel(x, weight, bias, ids_sorted):
    in_maps, C, counts, starts = _prepare(x, weight, bias, ids_sorted)
    res = _run_spmd(in_maps, C)
    return _assemble(res.results, counts, starts)


# revision 31
# speedup vs baseline: 1.0255x; 1.0015x over previous
"""MoE grouped linear (DMoELinear) on 8 Trainium2 NeuronCores.

Expert-parallel sharding: tokens are sorted by expert id, so expert e's
tokens form one contiguous slice. Core e receives expert e's tokens
(padded to a uniform capacity C = max group size, so all cores run one
SPMD NEFF), expert e's weight and bias, and computes
    yT_e = (x_e @ W_e.T).T.bf16 + b_e.bf16
with the weight block as the stationary matmul operand and tokens as
the moving free dim (so C needs no 128-alignment). The weight is laid
out db-major ([16 output blocks] x [8 k-tiles] x [128k x 128dout]) so
DMA arrival order matches PE consumption order. The bias add fuses into
the PSUM-evacuation op as a per-partition scalar. The host does all
routing/gather in numpy.
"""

import numpy as np
import ml_dtypes

N_TOK, D_IN, D_OUT, N_EXP = 8192, 1024, 2048, 8
N_CORES = 8
P = 128
NFREE = 512  # max matmul moving free dim (one PSUM bank of f32)

BF16 = ml_dtypes.bfloat16

_nc_cache: dict[int, object] = {}


def _chunks(C):
    out = []
    off = 0
    while off < C:
        cw = min(NFREE, C - off)
        out.append((off, cw))
        off += cw
    return out


def _build_bass(C: int):
    """Emit the per-core Bass/Tile kernel for token capacity C."""
    import concourse.bass as bass  # noqa: F401  (registers engines)
    import concourse.mybir as mybir
    import concourse.tile as tile
    from concourse import bacc

    dt = mybir.dt
    KT = D_IN // P      # 8 contraction tiles
    DB = D_OUT // P     # 16 output-row blocks
    chunks = _chunks(C)

    nc = bacc.Bacc("TRN2", target_bir_lowering=False)

    # x: [ki][128k x C] halves split across both HWDGE rings.
    xT_d = nc.dram_tensor("xT", [D_IN, C], dt.bfloat16, kind="ExternalInput")
    # w: db-major, partition-contiguous: row (db*128+p) holds all KT
    # 128-wide k-slices for output block db, SBUF partition p.
    w_d = nc.dram_tensor("wdb", [DB * P, KT * P], dt.bfloat16, kind="ExternalInput")
    bias_d = nc.dram_tensor("biasp", [P, DB], dt.float32, kind="ExternalInput")
    y_d = nc.dram_tensor("yT", [D_OUT, C], dt.bfloat16, kind="ExternalOutput")

    half = C // 2

    with tile.TileContext(nc) as tc:
        with (
            tc.tile_pool(name="persist", bufs=1) as ppool,
            tc.tile_pool(name="yout", bufs=3) as ypool,
            tc.tile_pool(name="psum", bufs=8, space="PSUM") as pspool,
        ):
            # DMA emission order = PE consumption order: w_db0/w_db1 lead
            # the two HWDGE rings, then x k-tiles (halves split across
            # rings), then the remaining w blocks alternating.
            w_tiles = [
                ppool.tile([P, KT * P], dt.bfloat16, name=f"w{db}", tag=f"w{db}")
                for db in range(DB)
            ]

            def w_dma(db):
                eng = nc.sync if db % 2 == 0 else nc.scalar
                eng.dma_start(w_tiles[db][:], w_d[db * P:(db + 1) * P, :])

            # Critical path to the first matmul (db0, k0, chunk0): its two
            # operand slices go first as small DMAs (earlier last-byte =>
            # earlier completion receipt), then the rest in consumption
            # order interleaved across the two rings.
            x_tiles = [
                ppool.tile([P, C], dt.bfloat16, name=f"x{ki}", tag=f"x{ki}")
                for ki in range(KT)
            ]
            nc.sync.dma_start(w_tiles[0][:, :P], w_d[:P, :P])
            nc.sync.dma_start(x_tiles[0][:, :half], xT_d[:P, :half])
            nc.sync.dma_start(w_tiles[0][:, P:], w_d[:P, P:])
            w_dma(1)  # scalar ring
            nc.scalar.dma_start(x_tiles[0][:, half:], xT_d[:P, half:])
            for ki in range(1, KT):
                nc.sync.dma_start(
                    x_tiles[ki][:, :half], xT_d[ki * P:(ki + 1) * P, :half]
                )
                nc.scalar.dma_start(
                    x_tiles[ki][:, half:], xT_d[ki * P:(ki + 1) * P, half:]
                )
            w_dma(2)
            w_dma(3)
            bt = ppool.tile([P, DB], dt.float32, name="bias", tag="bias")
            nc.sync.dma_start(bt[:], bias_d[:])
            for db in range(4, DB):
                w_dma(db)

            # Warm the PE's HAM clock gate with dummy matmuls on a zeroed
            # scratch tile while the input DMAs stream in (~3.4us of PE
            # activity flips the clock gate from 1.2GHz to 2.4GHz). Small
            # N=128 matmuls so real work waits at most ~110ns once its
            # inputs land.
            warm = ppool.tile([P, P], dt.bfloat16, name="warm", tag="warm")
            nc.vector.memset(warm[:], 0.0)
            wps = pspool.tile([P, P], dt.float32, name="wps", tag="ps")
            for _ in range(44):
                nc.tensor.matmul(wps[:], warm[:], warm[:], start=True, stop=True)

            all_psums = {}

            def alloc_psums(db):
                all_psums[db] = [
                    pspool.tile([P, NFREE], dt.float32, name=f"ps{db}_{j}", tag="ps")
                    for j in range(len(chunks))
                ]

            def emit_mms(db, ki):
                lhsT = w_tiles[db][:, ki * P:(ki + 1) * P]
                for j, (off, cw) in enumerate(chunks):
                    nc.tensor.matmul(
                        all_psums[db][j][:, :cw],
                        lhsT,
                        x_tiles[ki][:, off:off + cw],
                        start=(ki == 0),
                        stop=(ki == KT - 1),
                    )

            # First two db blocks interleaved by k-step: during the x DMA
            # trickle the PE consumes each freshly-landed x tile twice,
            # staying busier than a single db's worth of work would.
            alloc_psums(0)
            alloc_psums(1)
            for ki in range(KT):
                emit_mms(0, ki)
                emit_mms(1, ki)

            ep = 0
            for db in range(DB):
                if db >= 2:
                    alloc_psums(db)
                    for ki in range(KT):
                        emit_mms(db, ki)
                psums = all_psums[db]
                ysb = ypool.tile([P, C], dt.bfloat16, name="ysb", tag="ysb")
                bias_col = bt[:, db:db + 1]
                for j, (off, cw) in enumerate(chunks):
                    # fused PSUM->bf16 cast + per-partition bias add.
                    # Prologue/tail dbs: split each chunk across ACT and
                    # DVE in parallel, halving the latency until the PSUM
                    # bank frees (prologue: unblocks db3's bank rotation)
                    # resp. the y piece can ship (tail). Middle dbs keep
                    # the plain alternation that paces the steady state.
                    if (db < 2 or db >= DB - 2) and cw > 64:
                        h = (cw // 2 + 1) & ~1
                        nc.scalar.add(
                            ysb[:, off:off + h], psums[j][:, :h], bias_col
                        )
                        nc.vector.tensor_scalar_add(
                            ysb[:, off + h:off + cw], psums[j][:, h:cw], bias_col
                        )
                    elif ep % 2 == 0:
                        nc.scalar.add(ysb[:, off:off + cw], psums[j][:, :cw], bias_col)
                    else:
                        nc.vector.tensor_scalar_add(
                            ysb[:, off:off + cw], psums[j][:, :cw], bias_col
                        )
                    ep += 1
                if db >= DB - 2:
                    # tail blocks: per-chunk DMAs on alternating rings so the
                    # final transfer (smallest chunk) lands ASAP
                    for j, (off, cw) in enumerate(chunks):
                        eng = nc.sync if (db + j) % 2 == 0 else nc.scalar
                        eng.dma_start(
                            y_d[db * P:(db + 1) * P, off:off + cw],
                            ysb[:, off:off + cw],
                        )
                else:
                    eng = nc.sync if db % 2 == 0 else nc.scalar
                    eng.dma_start(y_d[db * P:(db + 1) * P, :], ysb[:])

    nc.compile()
    return nc


def _run_spmd(in_maps, C, trace=False, trace_cores=None):
    from concourse.bass_utils import run_bass_kernel_spmd

    nc = _nc_cache.get(C)
    if nc is None:
        nc = _build_bass(C)
        _nc_cache[C] = nc
    return run_bass_kernel_spmd(
        nc,
        in_maps,
        core_ids=list(range(N_CORES)),
        trace=trace,
        trace_cores=trace_cores,
    )


def _prepare(x, weight, bias, ids_sorted):
    """Host-side routing: returns (in_maps, C, counts, starts)."""
    x = np.asarray(x)
    weight = np.asarray(weight)
    bias = np.asarray(bias)
    ids = np.asarray(ids_sorted)

    counts = np.bincount(ids, minlength=N_EXP).astype(np.int64)
    starts = np.zeros(N_EXP, dtype=np.int64)
    starts[1:] = np.cumsum(counts)[:-1]
    C = max(int(counts.max()), 2)
    C += C % 2  # keep the half-split even

    xb = x.astype(BF16)
    in_maps = []
    for e in range(N_EXP):
        n_e = int(counts[e])
        xeT = np.zeros((D_IN, C), dtype=BF16)
        if n_e:
            xeT[:, :n_e] = xb[starts[e]:starts[e] + n_e].T
        # db-major weight: row (db*128+p) = [w.T[kt*128+p, db*128+d] for kt, d]
        weT = weight[e].T.astype(BF16)  # [d_in, d_out]
        wdb = np.ascontiguousarray(
            weT.reshape(D_IN // P, P, D_OUT // P, P).transpose(2, 1, 0, 3)
        ).reshape(D_OUT, D_IN)
        bp = np.ascontiguousarray(
            bias[e].astype(BF16).astype(np.float32).reshape(D_OUT // P, P).T
        )
        in_maps.append({"xT": xeT, "wdb": wdb, "biasp": bp})
    return in_maps, C, counts, starts


def _assemble(results, counts, starts):
    out = np.empty((N_TOK, D_OUT), dtype=BF16)
    for e in range(N_EXP):
        n_e = int(counts[e])
        if n_e:
            out[starts[e]:starts[e] + n_e] = results[e]["yT"][:, :n_e].T
    return out


def kernel(x, weight, bias, ids_sorted):
    in_maps, C, counts, starts = _prepare(x, weight, bias, ids_sorted)
    res = _run_spmd(in_maps, C)
    return _assemble(res.results, counts, starts)
